# revision 1
# baseline (speedup 1.0000x reference)
"""Trainium2 Bass kernel for the lipsnet CustomModel problem.

Math: the reference computes, per sample,
    jac_norm = ||D3 W3 D2 W2 D1 W1||_F      (Di = diag(relu'(pi)))
    out = tanh(k_out * f_out / (jac_norm + 1e-4))
Key identity used here:  with G = W1 W1^T = L L^T (host eigen factorization),
    ||D3 W3 D2 W2 D1 W1||_F^2 = ||D3 W3 D2 W2 D1 L||_F^2
                              = sum_c || D3 W3 D2 (M_c @ d1) ||^2
where M_c[j,l] = W2[j,l] * L[l,c] are 85 host-precomputed stationary
matrices and d1/d2/d3 are the per-sample binary relu masks.  Every
per-sample 85x85x85 contraction becomes a stationary-weight matmul with
the mask tensor [85, S] as the moving operand, so the TensorEngine does
all the heavy lifting; the only full-size elementwise work per c is one
DVE mask-multiply and one ACT square.  The sum over c of squares is
accumulated on the TensorEngine itself via an identity-matmul into a
persistent PSUM tile.

Sharding: pure data parallel over the batch dim, 8 NeuronCores, weights
replicated.  kernel() takes FULL inputs and returns the FULL output.
"""

import os
from contextlib import ExitStack

import numpy as np

import concourse.bass as bass
import concourse.bacc as bacc
import concourse.mybir as mybir
import concourse.tile as tile

F32 = mybir.dt.float32
AF = mybir.ActivationFunctionType
OP = mybir.AluOpType

B = 8192
OBS = 64
ACTD = 16
H = 128
COMP = 85
KS = 32
NCORES = 8
S = B // NCORES        # 1024 samples per core
NB = S // 128          # 8 sample blocks of 128
CH = 512               # matmul moving-operand chunk (one PSUM bank of f32)
EPS = 1e-4

# name -> (shape, bf16?) of every replicated weight, packed host-side into
# two [128, N] arrays (one f32, one bf16) so the kernel needs just 2 DMAs
_WSLOTS = {
    "ow1T": ([OBS, H], 0), "ob1": ([H, 1], 0), "ow2T": ([H, H], 0),
    "ob2": ([H, 1], 0), "aw1T": ([ACTD, H], 0), "ab1": ([H, 1], 0),
    "aw2T": ([H, H], 0), "ab2": ([H, 1], 0),
    "kw1Ta": ([H, KS], 0), "kw1Tb": ([H, KS], 0), "kb1": ([KS, 1], 0),
    "kw2T": ([KS, KS // 2], 0), "kb2": ([KS // 2, 1], 0),
    "kw3T": ([KS // 2, 1], 0), "kb3": ([1, 1], 0),
    "mw1Ta": ([H, COMP], 0), "mw1Tb": ([H, COMP], 0), "mb1": ([COMP, 1], 0),
    "mw2T": ([COMP, COMP], 0), "mb2": ([COMP, 1], 0),
    "mw3T": ([COMP, COMP], 0), "mb3": ([COMP, 1], 0),
    "ones": ([COMP, 1], 0), "iden": ([H, H], 0),
    "mall": ([COMP, COMP * COMP], 1), "mw3Tb": ([COMP, COMP], 1),
    "idenb": ([COMP, COMP], 1), "onesb": ([COMP, 1], 1),
}
_OFFS = {}
_NCOLS = [0, 0]
for _n, (_shp, _b) in _WSLOTS.items():
    _OFFS[_n] = _NCOLS[_b]
    _NCOLS[_b] += _shp[1]

def host_prep(inputs):
    """Host-side weight preprocessing + packing (pure numpy, all tiny)."""
    import ml_dtypes
    f = lambda a: np.ascontiguousarray(np.asarray(a, dtype=np.float32))
    W1, W2, W3 = f(inputs["mw1"]), f(inputs["mw2"]), f(inputs["mw3"])
    G = (W1 @ W1.T).astype(np.float64)
    lam, U = np.linalg.eigh(G)
    L = (U * np.sqrt(np.clip(lam, 0.0, None))).astype(np.float32)  # G = L L^T
    # mall[l, c*85+j] = W2[j, l] * L[l, c]   (stage-1 stationary lhsT per c)
    mall = (W2.T[:, None, :] * L[:, :, None]).reshape(COMP, COMP * COMP)
    vals = {
        "ow1T": f(inputs["ow1"]).T, "ob1": f(inputs["ob1"]).reshape(H, 1),
        "ow2T": f(inputs["ow2"]).T, "ob2": f(inputs["ob2"]).reshape(H, 1),
        "aw1T": f(inputs["aw1"]).T, "ab1": f(inputs["ab1"]).reshape(H, 1),
        "aw2T": f(inputs["aw2"]).T, "ab2": f(inputs["ab2"]).reshape(H, 1),
        "kw1Ta": f(inputs["kw1"]).T[:H], "kw1Tb": f(inputs["kw1"]).T[H:],
        "kb1": f(inputs["kb1"]).reshape(KS, 1),
        "kw2T": f(inputs["kw2"]).T, "kb2": f(inputs["kb2"]).reshape(KS // 2, 1),
        "kw3T": f(inputs["kw3"]).T, "kb3": f(inputs["kb3"]).reshape(1, 1),
        "mw1Ta": W1.T[:H], "mw1Tb": W1.T[H:],
        "mb1": f(inputs["mb1"]).reshape(COMP, 1),
        "mw2T": W2.T, "mb2": f(inputs["mb2"]).reshape(COMP, 1),
        "mw3T": W3.T, "mb3": f(inputs["mb3"]).reshape(COMP, 1),
        "ones": np.ones((COMP, 1), np.float32),
        "iden": np.eye(H, dtype=np.float32),
        "mall": mall, "mw3Tb": W3.T,
        "idenb": np.eye(COMP, dtype=np.float32),
        "onesb": np.ones((COMP, 1), np.float32),
    }
    packs = [np.zeros((128, _NCOLS[0]), np.float32),
             np.zeros((128, _NCOLS[1]), ml_dtypes.bfloat16)]
    for n, (shp, b) in _WSLOTS.items():
        o = _OFFS[n]
        packs[b][:shp[0], o:o + shp[1]] = vals[n]
    return {"wpack32": packs[0], "wpack16": packs[1]}


def build_nc(reps=1):
    nc = bacc.Bacc()

    obs_d = nc.declare_dram_parameter("obs", [S, OBS], F32, isOutput=False)
    act_d = nc.declare_dram_parameter("action", [S, ACTD], F32, isOutput=False)
    BF16 = mybir.dt.bfloat16
    wp32_d = nc.declare_dram_parameter("wpack32", [128, _NCOLS[0]], F32,
                                       isOutput=False)
    wp16_d = nc.declare_dram_parameter("wpack16", [128, _NCOLS[1]], BF16,
                                       isOutput=False)
    tick_d = nc.declare_dram_parameter("tick", [1, 1], F32, isOutput=False)
    out_d = nc.declare_dram_parameter("out", [S, COMP], F32, isOutput=True)

    with tile.TileContext(nc) as tc, ExitStack() as ctx:
        wp = ctx.enter_context(tc.tile_pool(name="weights", bufs=1))
        ap = ctx.enter_context(tc.tile_pool(name="acts", bufs=1))
        zp = ctx.enter_context(tc.tile_pool(name="zbuf", bufs=4))
        sqp = ctx.enter_context(tc.tile_pool(name="sqbuf", bufs=4))
        outp = ctx.enter_context(tc.tile_pool(name="outbuf", bufs=3))
        smp = ctx.enter_context(tc.tile_pool(name="small", bufs=16))
        psA = ctx.enter_context(tc.tile_pool(name="psA", bufs=3, space="PSUM"))
        psC = ctx.enter_context(tc.tile_pool(name="psC", bufs=1, space="PSUM"))

        # ---- load weights (2 packed DMAs), expose per-weight slice views ----
        wp32 = wp.tile([128, _NCOLS[0]], F32, tag="wp32", name="wp32")
        wp16 = wp.tile([128, _NCOLS[1]], BF16, tag="wp16", name="wp16")
        nc.sync.dma_start(wp32[:], wp32_d[:])
        w = {}
        for name, (shp, b) in _WSLOTS.items():
            o = _OFFS[name]
            w[name] = (wp16 if b else wp32)[0:shp[0], o:o + shp[1]]

        tick_sb = wp.tile([1, 1], F32, tag="tick_sb", name="tick_sb")
        nc.sync.dma_start(tick_sb[:], tick_d[:])

        # ---- load + transpose obs/action into [feat, S] layout ----
        for _rep in range(reps):
            obs_sb = ap.tile([128, NB, OBS], F32, tag="obs_sb")
            act_sb = ap.tile([128, NB, ACTD], F32, tag="act_sb")
            for nb in range(NB):
                nc.sync.dma_start(obs_sb[:, nb, :], obs_d[nb * 128:(nb + 1) * 128, :])
                nc.sync.dma_start(act_sb[:, nb, :], act_d[nb * 128:(nb + 1) * 128, :])
            # collapse the many DMA-queue semaphores into one barrier so no
            # matmul needs more than one sync wait (walrus S3_LW limit)
            tc.strict_bb_all_engine_barrier()
            # the big bf16 pack (stage-1 matrices) is only needed at J-loop
            # start; issued after the barrier so the forward overlaps it
            nc.sync.dma_start(wp16[:], wp16_d[:])

            obst = ap.tile([OBS, S], F32, tag="obst")
            actt = ap.tile([ACTD, S], F32, tag="actt")
            for nb in range(NB):
                pt = psA.tile([OBS, 128], F32, tag="a")
                nc.tensor.transpose(pt[:], obs_sb[:, nb, :], w["iden"][:])
                nc.vector.tensor_copy(obst[:, nb * 128:(nb + 1) * 128], pt[:])
                pt2 = psA.tile([ACTD, 128], F32, tag="a")
                nc.tensor.transpose(pt2[:], act_sb[:, nb, :], w["iden"][:])
                nc.vector.tensor_copy(actt[:, nb * 128:(nb + 1) * 128], pt2[:])

            # ---- forward layers ([feat, S], chunked matmuls + fused ACT) ----
            def layer(dst, dst_sl, terms, bias, func, p):
                # dst[dst_sl] = func(sum_i lhsT_i.T @ rhs_i + bias), chunked over S
                m = dst.shape[-1] if dst_sl is None else None
                for ch in range(S // CH):
                    sl = slice(ch * CH, (ch + 1) * CH)
                    pt = p.tile([terms[0][0].shape[-1], CH], F32, tag="a", name="pt")
                    n = len(terms)
                    for i, (lhsT, rhs) in enumerate(terms):
                        nc.tensor.matmul(pt[:], lhsT[:], rhs[:, sl],
                                         start=(i == 0), stop=(i == n - 1))
                    dsl = dst[:, sl] if dst_sl is None else dst[dst_sl, sl]
                    if func == AF.Relu:
                        nc.vector.tensor_scalar(out=dsl, in0=pt[:], scalar1=bias[:],
                                                scalar2=0.0, op0=OP.add, op1=OP.max)
                    else:
                        nc.scalar.activation(dsl, pt[:], func, bias=bias[:])

            oh1 = ap.tile([H, S], F32, tag="oh1")
            layer(oh1, None, [(w["ow1T"], obst)], w["ob1"], AF.Relu, psA)
            of = ap.tile([H, S], F32, tag="of")
            layer(of, None, [(w["ow2T"], oh1)], w["ob2"], AF.Relu, psA)
            ah1 = ap.tile([H, S], F32, tag="ah1")
            layer(ah1, None, [(w["aw1T"], actt)], w["ab1"], AF.Relu, psA)
            af = ap.tile([H, S], F32, tag="af")
            layer(af, None, [(w["aw2T"], ah1)], w["ab2"], AF.Relu, psA)

            k1 = ap.tile([KS, S], F32, tag="k1")
            layer(k1, None, [(w["kw1Ta"], of), (w["kw1Tb"], af)], w["kb1"], AF.Tanh, psA)
            k2 = ap.tile([KS // 2, S], F32, tag="k2")
            layer(k2, None, [(w["kw2T"], k1)], w["kb2"], AF.Tanh, psA)

            # k_out = softplus(kw3 @ k2 + kb3) = ln(1 + exp(.)) via Exp then Ln(x+1)
            kexp = ap.tile([1, S], F32, tag="kexp")
            layer(kexp, None, [(w["kw3T"], k2)], w["kb3"], AF.Exp, psA)
            kout = ap.tile([1, S], F32, tag="kout")
            nc.scalar.activation(kout[:], kexp[:], AF.Ln, bias=1.0)

            h1 = ap.tile([COMP, S], F32, tag="h1")
            layer(h1, None, [(w["mw1Ta"], of), (w["mw1Tb"], af)], w["mb1"], AF.Relu, psA)
            d1 = ap.tile([COMP, S], BF16, tag="d1")
            nc.vector.tensor_scalar(out=d1[:], in0=h1[:], scalar1=0.0, scalar2=None,
                                    op0=OP.is_gt)
            h2 = ap.tile([COMP, S], F32, tag="h2")
            layer(h2, None, [(w["mw2T"], h1)], w["mb2"], AF.Relu, psA)
            d2 = ap.tile([COMP, S], F32, tag="d2")
            nc.vector.tensor_scalar(out=d2[:], in0=h2[:], scalar1=0.0, scalar2=None,
                                    op0=OP.is_gt)
            fout = ap.tile([COMP, S], F32, tag="fout")
            layer(fout, None, [(w["mw3T"], h2)], w["mb3"], AF.Relu, psA)
            d3 = ap.tile([COMP, S], F32, tag="d3")
            nc.vector.tensor_scalar(out=d3[:], in0=fout[:], scalar1=0.0,
                                    scalar2=None, op0=OP.is_gt)

            # ---- Jacobian-norm loop over the 85 columns of L ----
            # bf16 identity for the accumulate-matmul (fp32 matmuls lower to
            # HI/LO pairs that break inside an interleaved accumulation group)
            idenb = w["idenb"]
            accp = psC.tile([COMP, S], F32, tag="c")   # persistent PSUM accumulator
            ACCs = ap.tile([COMP, S], F32, tag="ACCs")  # SBUF spill of acc groups
            GRP = 28   # accumulation-group length (bounded for HW robustness)
            acc_n = [0]

            def acc_mm(sq):
                n = acc_n[0]
                for ch in range(S // CH):
                    sl = slice(ch * CH, (ch + 1) * CH)
                    nc.tensor.matmul(accp[:, sl], idenb[:], sq[:, sl],
                                     start=(n % GRP == 0),
                                     stop=(n % GRP == GRP - 1 or n == COMP - 1),
                                     skip_group_check=True)
                acc_n[0] = n + 1
                if n % GRP == GRP - 1 or n == COMP - 1:
                    if n < GRP:
                        nc.vector.tensor_copy(ACCs[:], accp[:])
                    else:
                        nc.vector.tensor_tensor(ACCs[:], accp[:], ACCs[:], OP.add)

            tc.strict_bb_all_engine_barrier()
            # software pipeline: py prefetched one c ahead of the DVE mask,
            # squares accumulated two c behind, so PE never heads-of-line
            # blocks the mask -> pr -> py -> mask cycle
            pys = {}

            def emit_py(c):
                t = psA.tile([COMP, S], F32, tag="a", name="py")
                for ch in range(S // CH):
                    sl = slice(ch * CH, (ch + 1) * CH)
                    nc.tensor.matmul(t[:, sl], w["mall"][:, c * COMP:(c + 1) * COMP],
                                     d1[:, sl], start=True, stop=True)
                pys[c] = t

            emit_py(0)
            pend = []
            for c in range(COMP):
                z = zp.tile([COMP, S], BF16, tag="z")
                nc.vector.tensor_tensor(z[:], pys.pop(c)[:], d2[:], OP.mult)
                if c + 1 < COMP:
                    emit_py(c + 1)
                if len(pend) == 2:
                    acc_mm(pend.pop(0))
                pr = psA.tile([COMP, S], F32, tag="a", name="pr")
                for ch in range(S // CH):
                    sl = slice(ch * CH, (ch + 1) * CH)
                    nc.tensor.matmul(pr[:, sl], w["mw3Tb"][:], z[:, sl],
                                     start=True, stop=True)
                sq = sqp.tile([COMP, S], BF16, tag="sq")
                nc.scalar.square(sq[:], pr[:])
                pend.append(sq)
            acc_mm(pend.pop(0))
            acc_mm(pend.pop(0))

            # ---- finale: jn2 = ones^T (d3 * acc); out = tanh(kout*fout/(sqrt+eps)) ----
            am = zp.tile([COMP, S], BF16, tag="am")
            nc.vector.tensor_tensor(am[:], ACCs[:], d3[:], OP.mult)
            pj = psA.tile([1, S], F32, tag="a", name="pj")
            pj_lhs = w["onesb"]
            for ch in range(S // CH):
                sl = slice(ch * CH, (ch + 1) * CH)
                nc.tensor.matmul(pj[:, sl], pj_lhs[:], am[:, sl],
                                 start=True, stop=True)
            jn2 = ap.tile([1, S], F32, tag="jn2")
            nc.scalar.copy(jn2[:], pj[:])

            tc.strict_bb_all_engine_barrier()

            # batch the per-sample scale: transpose jn2/kout for all blocks
            # into one [128, 2*NB] tile, then one sqrt + vector recip pass
            pjk = psA.tile([128, 2 * NB], F32, tag="a", name="pjk")
            for nb in range(NB):
                sl = slice(nb * 128, (nb + 1) * 128)
                nc.tensor.transpose(pjk[:, nb:nb + 1], jn2[:, sl], w["iden"][:1, :1])
                nc.tensor.transpose(pjk[:, NB + nb:NB + nb + 1], kout[:, sl],
                                    w["iden"][:1, :1])
            den = smp.tile([128, NB], F32, tag="den")
            nc.scalar.activation(den[:], pjk[:, 0:NB], AF.Sqrt)
            rec = smp.tile([128, NB], F32, tag="rec")
            nc.vector.tensor_scalar_add(rec[:], den[:], EPS)
            nc.vector.reciprocal(rec[:], rec[:])
            scl = smp.tile([128, NB], F32, tag="scl")
            nc.vector.tensor_tensor(scl[:], rec[:], pjk[:, NB:2 * NB], OP.mult)
            for nb in range(NB):
                sl = slice(nb * 128, (nb + 1) * 128)
                pt = psA.tile([128, COMP], F32, tag="a", name="ptf")
                nc.tensor.transpose(pt[:], fout[:, sl], w["iden"][:COMP, :COMP])
                ot = outp.tile([128, COMP], F32, tag="ot")
                nc.scalar.activation(ot[:], pt[:], AF.Tanh, scale=scl[:, nb:nb + 1])
                nc.sync.dma_start(out_d[sl, :], ot[:])

    return nc


_NC = None


def _get_nc():
    global _NC
    if _NC is None:
        _NC = build_nc()
        _NC.finalize()
    return _NC


def make_in_maps(inputs):
    w = host_prep(inputs)
    obs = np.ascontiguousarray(np.asarray(inputs["obs"], np.float32))
    act = np.ascontiguousarray(np.asarray(inputs["action"], np.float32))
    in_maps = []
    for i in range(NCORES):
        m = dict(w)
        m["obs"] = np.ascontiguousarray(obs[i * S:(i + 1) * S])
        m["action"] = np.ascontiguousarray(act[i * S:(i + 1) * S])
        m["tick"] = np.zeros((1, 1), np.float32)
        in_maps.append(m)
    return in_maps


def kernel(**inputs):
    from concourse.bass_utils import run_bass_kernel_spmd

    nc = _get_nc()
    in_maps = make_in_maps(inputs)
    res = run_bass_kernel_spmd(nc, in_maps, core_ids=list(range(NCORES)))
    return np.concatenate([r["out"] for r in res.results], axis=0)



# revision 42
# speedup vs baseline: 1.1404x; 1.1404x over previous
"""Trainium2 Bass kernel for the lipsnet CustomModel problem (v2).

Math: per sample,
    jac_norm = ||D3 W3 D2 W2 D1 W1||_F      (Di = diag(relu'(pi)))
    out = tanh(k_out * f_out / (jac_norm + 1e-4))
With G = W1 W1^T = L L^T (host eigen factorization),
    ||D3 W3 D2 W2 D1 W1||_F^2 = sum_c || W3 (d2 o (W2 (l_c o d1))) o d3 ||^2
so the J-loop runs 85 stationary-weight matmuls with the binary masks as
moving operands.  v2 structural changes over the v1 baseline:
  - batched input/output DMAs (2 in, 1 out instead of 24)
  - J-loop matmuls use N=1024 bf16 moving operands (1 instr per stage)
  - square results written into 128-partition-packed tiles so the
    accumulate matmuls shrink from 85 to 57 (K=128 instead of 85)
  - accumulation kept in PSUM the whole loop (single 57-matmul group),
    final d3-mask reads PSUM directly -- no SBUF spills
  - activation table sets sequenced so no LoadActFuncSet lands after the
    J-loop starts (dummy tanh forces a square+tanh set before c=0)
  - sqrt via DVE Newton iteration (Quake-style init) instead of an ACT
    table swap in the tail
  - fout pre-transposed during the J-loop; single fused output DMA

Sharding: pure data parallel over batch, 8 cores, weights replicated.
"""

import os
from contextlib import ExitStack

import numpy as np

import concourse.bass as bass
import concourse.bacc as bacc
import concourse.mybir as mybir
import concourse.tile as tile

F32 = mybir.dt.float32
BF16 = mybir.dt.bfloat16
I32 = mybir.dt.int32
AF = mybir.ActivationFunctionType
OP = mybir.AluOpType

B = 8192
OBS = 64
ACTD = 16
OA = OBS + ACTD        # 80
H = 128
COMP = 85
KS = 32
NCORES = 8
S = B // NCORES        # 1024 samples per core
NB = S // 128          # 8 sample blocks
NROWS = COMP * COMP    # 7225 packed (c, m) rows
NT = (NROWS + 127) // 128   # 57 sq pack tiles
CH_F = 512             # f32 moving-operand chunk
EPS = 1e-4

CH_J = 512             # J-loop moving chunk (ISA caps AP dims at 512)
USE_QUAKE = True       # DVE rsqrt instead of ACT Sqrt table swap
ACC_DR = True          # fp8 DoubleRow accumulate (2 c's per matmul)
SQS = 16.0             # fp8 sq scale: sq' = (SQS*pr)^2, jn2' = SQS^2 * jn2
MAGIC = 0x5F3759DF
NEWTON = 2

_WSLOTS32 = {
    "ow1T": [OBS, H], "ob1": [H, 1], "ow2T": [H, H], "ob2": [H, 1],
    "aw1T": [ACTD, H], "ab1": [H, 1], "aw2T": [H, H], "ab2": [H, 1],
    "kw1Ta": [H, KS], "kw1Tb": [H, KS], "kb1": [KS, 1],
    "kw2T": [KS, KS // 2], "kb2": [KS // 2, 1],
    "kw3T": [KS // 2, 1], "kb3": [1, 1],
    "mw1Ta": [H, COMP], "mw1Tb": [H, COMP], "mb1": [COMP, 1],
    "mw2T": [COMP, COMP], "mb2": [COMP, 1],
    "mw3T": [COMP, COMP], "mb3": [COMP, 1],
    "iden": [H, H],
}
_WSLOTS16 = {
    "mw3Tb": [COMP, COMP], "onesb": [COMP, 1], "idenb": [COMP, COMP],
    "kw1Ta16": [H, KS], "kw1Tb16": [H, KS], "kw2T16": [KS, KS // 2],
    "kw3T16": [KS // 2, 1],
    "mall": [COMP, COMP * COMP],
}
_OFFS32, _OFFS16 = {}, {}
_NC32 = 0
for _n, _shp in _WSLOTS32.items():
    _OFFS32[_n] = _NC32
    _NC32 += _shp[1]
_NC16 = 0
for _n, _shp in _WSLOTS16.items():
    _OFFS16[_n] = _NC16
    _NC16 += _shp[1]



def host_prep(inputs):
    """Host-side weight preprocessing + packing (pure numpy, all tiny)."""
    import ml_dtypes
    f = lambda a: np.ascontiguousarray(np.asarray(a, dtype=np.float32))
    W1, W2, W3 = f(inputs["mw1"]), f(inputs["mw2"]), f(inputs["mw3"])
    G = (W1 @ W1.T).astype(np.float64)
    lam, U = np.linalg.eigh(G)
    L = (U * np.sqrt(np.clip(lam, 0.0, None))).astype(np.float32)  # G = L L^T
    # mall[l, c*85+j] = W2[j, l] * L[l, c]
    mall = (W2.T[:, None, :] * L[:, :, None]).reshape(COMP, COMP * COMP)
    vals32 = {
        "ow1T": f(inputs["ow1"]).T, "ob1": f(inputs["ob1"]).reshape(H, 1),
        "ow2T": f(inputs["ow2"]).T, "ob2": f(inputs["ob2"]).reshape(H, 1),
        "aw1T": f(inputs["aw1"]).T, "ab1": f(inputs["ab1"]).reshape(H, 1),
        "aw2T": f(inputs["aw2"]).T, "ab2": f(inputs["ab2"]).reshape(H, 1),
        "kw1Ta": f(inputs["kw1"]).T[:H], "kw1Tb": f(inputs["kw1"]).T[H:],
        "kb1": f(inputs["kb1"]).reshape(KS, 1),
        "kw2T": f(inputs["kw2"]).T, "kb2": f(inputs["kb2"]).reshape(KS // 2, 1),
        "kw3T": f(inputs["kw3"]).T, "kb3": f(inputs["kb3"]).reshape(1, 1),
        "mw1Ta": W1.T[:H], "mw1Tb": W1.T[H:],
        "mb1": f(inputs["mb1"]).reshape(COMP, 1),
        "mw2T": W2.T, "mb2": f(inputs["mb2"]).reshape(COMP, 1),
        "mw3T": W3.T, "mb3": f(inputs["mb3"]).reshape(COMP, 1),
        "iden": np.eye(H, dtype=np.float32),
    }
    vals16 = {
        "mw3Tb": W3.T, "onesb": np.ones((COMP, 1), np.float32),
        "idenb": np.eye(COMP, dtype=np.float32), "mall": mall,
        "kw1Ta16": f(inputs["kw1"]).T[:H], "kw1Tb16": f(inputs["kw1"]).T[H:],
        "kw2T16": f(inputs["kw2"]).T, "kw3T16": f(inputs["kw3"]).T,
    }
    p32 = np.zeros((128, _NC32), np.float32)
    for n, shp in _WSLOTS32.items():
        p32[:shp[0], _OFFS32[n]:_OFFS32[n] + shp[1]] = vals32[n]
    p16 = np.zeros((128, _NC16), ml_dtypes.bfloat16)
    for n, shp in _WSLOTS16.items():
        p16[:shp[0], _OFFS16[n]:_OFFS16[n] + shp[1]] = vals16[n]
    # fp8 pack: iden2[k, i, m] = (k == m), pair dim second for DoubleRow
    iden2 = np.zeros((128, 2, 96), np.float32)
    for k in range(COMP):
        iden2[k, 0, k] = 1.0
        iden2[k, 1, k] = 1.0
    p8 = iden2.reshape(128, 192).astype(ml_dtypes.float8_e4m3)
    return {"wpack32": p32, "wpack16": p16, "wpack8": p8}


def build_nc(reps=1):
    nc = bacc.Bacc()

    obs_d = nc.declare_dram_parameter("obs", [S, OBS], F32, isOutput=False)
    act_d = nc.declare_dram_parameter("action", [S, ACTD], F32, isOutput=False)
    wp32_d = nc.declare_dram_parameter("wpack32", [128, _NC32], F32,
                                       isOutput=False)
    wp16_d = nc.declare_dram_parameter("wpack16", [128, _NC16], BF16,
                                       isOutput=False)
    FP8 = mybir.dt.float8e4
    wp8_d = nc.declare_dram_parameter("wpack8", [128, 192], FP8,
                                      isOutput=False)
    out_d = nc.declare_dram_parameter("out", [S, COMP], F32, isOutput=True)

    with tile.TileContext(nc) as tc, ExitStack() as ctx:
        wp = ctx.enter_context(tc.tile_pool(name="weights", bufs=1))
        ap_ = ctx.enter_context(tc.tile_pool(name="acts", bufs=1))
        zp = ctx.enter_context(tc.tile_pool(name="zbuf", bufs=6))
        sqp = ctx.enter_context(tc.tile_pool(name="sqbuf", bufs=6))
        smp = ctx.enter_context(tc.tile_pool(name="small", bufs=1))
        outp = ctx.enter_context(tc.tile_pool(name="outbuf", bufs=1))
        psBig = ctx.enter_context(tc.tile_pool(name="psBig", bufs=2,
                                               space="PSUM"))
        psPr = ctx.enter_context(tc.tile_pool(name="psPr", bufs=2,
                                              space="PSUM"))
        psC = ctx.enter_context(tc.tile_pool(name="psC", bufs=1, space="PSUM"))

        wp32 = wp.tile([128, _NC32], F32, tag="wp32", name="wp32")
        wp16 = wp.tile([128, _NC16], BF16, tag="wp16", name="wp16")
        wp8 = wp.tile([128, 2, 96], FP8, tag="wp8", name="wp8")
        nc.sync.dma_start(wp32[:], wp32_d[:])
        nc.sync.dma_start(wp8[:], wp8_d[:])
        w = {}
        for n, shp in _WSLOTS32.items():
            w[n] = wp32[0:shp[0], _OFFS32[n]:_OFFS32[n] + shp[1]]
        for n, shp in _WSLOTS16.items():
            w[n] = wp16[0:shp[0], _OFFS16[n]:_OFFS16[n] + shp[1]]

        def jchunks():
            return [slice(i, i + CH_J) for i in range(0, S, CH_J)]

        for _rep in range(reps):
            # ---- inputs: 2 batched DMAs, obs+action side by side ----
            oa_sb = ap_.tile([128, NB, OA], F32, tag="oa_sb", name="oa_sb")
            nc.sync.dma_start(oa_sb[:, :, 0:OBS],
                              obs_d[:].rearrange("(nb p) f -> p nb f", p=128))
            nc.sync.dma_start(oa_sb[:, :, OBS:OA],
                              act_d[:].rearrange("(nb p) f -> p nb f", p=128))
            tc.strict_bb_all_engine_barrier()
            # big bf16 pack needed only from J-loop start; issue post-barrier
            nc.sync.dma_start(wp16[:], wp16_d[:])

            # ---- transpose obs|action into [feat, S] (8 combined blocks) ----
            oa_t = ap_.tile([OBS, S], F32, tag="oa_t", name="oa_t")
            at_t = ap_.tile([ACTD, S], F32, tag="at_t", name="at_t")
            for nb in range(NB):
                ptt = psBig.tile([OA, 128], F32, tag="big", name="ptt")
                nc.tensor.transpose(ptt[:], oa_sb[:, nb, :], w["iden"])
                sl = slice(nb * 128, (nb + 1) * 128)
                nc.scalar.copy(oa_t[:, sl], ptt[0:OBS, :])
                nc.scalar.copy(at_t[:, sl], ptt[OBS:OA, :])
            obst = oa_t[:]
            actt = at_t[:]

            # ---- forward layers (f32, [feat, S]) ----
            def layer_ps(terms):
                m = terms[0][0].shape[-1]
                pt = psBig.tile([m, S], F32, tag="big", name="lps")
                n = len(terms)
                for i, (lhsT, rhs) in enumerate(terms):
                    for c0 in range(0, S, CH_F):
                        sl = slice(c0, c0 + CH_F)
                        nc.tensor.matmul(pt[0:m, sl], lhsT, rhs[:, sl],
                                         start=(i == 0), stop=(i == n - 1))
                return pt

            def relu_dve(dst, pt, bias):
                nc.vector.tensor_scalar(out=dst, in0=pt, scalar1=bias,
                                        scalar2=0.0, op0=OP.add, op1=OP.max)

            oh1 = ap_.tile([H, S], F32, tag="oh1", name="oh1")
            pt = layer_ps([(w["ow1T"], obst)])
            nc.scalar.activation(oh1[:], pt[0:H, :], AF.Relu, bias=w["ob1"])
            ah1 = ap_.tile([H, S], F32, tag="ah1", name="ah1")
            pt = layer_ps([(w["aw1T"], actt)])
            nc.scalar.activation(ah1[:], pt[0:H, :], AF.Relu, bias=w["ab1"])
            of = ap_.tile([H, S], F32, tag="of", name="of")
            pt = layer_ps([(w["ow2T"], oh1)])
            relu_dve(of[:], pt[0:H, :], w["ob2"])
            af = ap_.tile([H, S], F32, tag="af", name="af")
            pt = layer_ps([(w["aw2T"], ah1)])
            relu_dve(af[:], pt[0:H, :], w["ab2"])

            # k-net runs on ACT/PE interleaved with the m-net chain so its
            # tanh/ln + table loads hide behind m-net matmul time
            of16 = ap_.tile([H, S], BF16, tag="of16", name="of16")
            nc.vector.tensor_copy(of16[:], of[:])
            af16 = ap_.tile([H, S], BF16, tag="af16", name="af16")
            nc.vector.tensor_copy(af16[:], af[:])
            k1 = ap_.tile([KS, S], BF16, tag="k1", name="k1")
            k1ps = psBig.tile([KS, S], F32, tag="big", name="k1ps")
            for sl in jchunks():
                nc.tensor.matmul(k1ps[0:KS, sl], w["kw1Ta16"], of16[:, sl],
                                 start=True, stop=False)
                nc.tensor.matmul(k1ps[0:KS, sl], w["kw1Tb16"], af16[:, sl],
                                 start=False, stop=True)
            nc.scalar.activation(k1[:], k1ps[0:KS, :], AF.Tanh, bias=w["kb1"])

            h1s = ap_.tile([COMP, S], F32, tag="h1s", name="h1s")
            d1 = ap_.tile([COMP, S], BF16, tag="d1", name="d1")
            pt = layer_ps([(w["mw1Ta"], of), (w["mw1Tb"], af)])
            relu_dve(h1s[:], pt[0:COMP, :], w["mb1"])
            nc.vector.tensor_scalar(out=d1[:], in0=h1s[:], scalar1=0.0,
                                    scalar2=None, op0=OP.is_gt)

            k2 = ap_.tile([KS // 2, S], BF16, tag="k2", name="k2")
            k2ps = psBig.tile([KS // 2, S], F32, tag="big", name="k2ps")
            for sl in jchunks():
                nc.tensor.matmul(k2ps[0:KS // 2, sl], w["kw2T16"], k1[:, sl],
                                 start=True, stop=True)
            nc.scalar.activation(k2[:], k2ps[0:KS // 2, :], AF.Tanh,
                                 bias=w["kb2"])

            h2s = ap_.tile([COMP, S], F32, tag="h2s", name="h2s")
            d2 = ap_.tile([COMP, S], F32, tag="d2", name="d2")
            pt = layer_ps([(w["mw2T"], h1s)])
            relu_dve(h2s[:], pt[0:COMP, :], w["mb2"])
            nc.vector.tensor_scalar(out=d2[:], in0=h2s[:], scalar1=0.0,
                                    scalar2=None, op0=OP.is_gt)

            kexp = ap_.tile([1, S], F32, tag="kexp", name="kexp")
            keps = psBig.tile([1, S], F32, tag="big", name="keps")
            for sl in jchunks():
                nc.tensor.matmul(keps[0:1, sl], w["kw3T16"], k2[:, sl],
                                 start=True, stop=True)
            nc.scalar.activation(kexp[:], keps[0:1, :], AF.Exp, bias=w["kb3"])
            kout = ap_.tile([1, S], F32, tag="kout", name="kout")
            nc.scalar.activation(kout[:], kexp[:], AF.Ln, bias=1.0)
            # dummy tanh: forces the act-table pass to load a tanh+square set
            # here, so the J-loop squares and the final tanh need no swap
            dum = smp.tile([1, 1], F32, tag="dum", name="dum")
            nc.scalar.activation(dum[:], kexp[0:1, 0:1], AF.Tanh)

            fouts = ap_.tile([COMP, S], F32, tag="fouts", name="fouts")
            d3 = ap_.tile([COMP, S], BF16, tag="d3", name="d3")
            pt = layer_ps([(w["mw3T"], h2s)])
            relu_dve(fouts[:], pt[0:COMP, :], w["mb3"])
            nc.vector.tensor_scalar(out=d3[:], in0=fouts[:], scalar1=0.0,
                                    scalar2=None, op0=OP.is_gt)

            # ---- fout pre-transpose (before J so the pool parity is clean)
            foutT = ap_.tile([128, NB, COMP], F32, tag="foutT", name="foutT")
            for nb in range(NB):
                ptf = psBig.tile([128, COMP], F32, tag="big", name="ptf")
                nc.tensor.transpose(ptf[:], fouts[:, nb * 128:(nb + 1) * 128],
                                    w["iden"][0:COMP, 0:COMP])
                nc.scalar.copy(foutT[:, nb, :], ptf[:])

            # ---- J-loop: 85 eigencolumns ----
            accp = psC.tile([96, S], F32, tag="c", name="accp")
            pys = {}

            def emit_py(c):
                t = psBig.tile([COMP, S], F32, tag="big", name="py")
                for sl in jchunks():
                    nc.tensor.matmul(t[0:COMP, sl],
                                     w["mall"][:, c * COMP:(c + 1) * COMP],
                                     d1[:, sl], start=True, stop=True)
                pys[c] = t

            emit_py(0)
            emit_py(1)
            NPAIR = COMP // 2          # 42 fp8 DoubleRow pair-accumulates
            pend = []
            acc_n = [0]

            def emit_acc_pair(sq2, last):
                # accp[0:96] += sum_i iden2[:, i, :].T-weighted sq2[:, i, :]
                for i0 in range(0, S, 512):
                    sl = slice(i0, i0 + 512)
                    nc.tensor.matmul(accp[0:96, sl], wp8[0:COMP, :, :],
                                     sq2[0:COMP, :, sl],
                                     start=(acc_n[0] == 0), stop=last,
                                     skip_group_check=True,
                                     perf_mode=mybir.MatmulPerfMode.DoubleRow)
                acc_n[0] += 1

            for c in range(COMP):
                if c + 2 < COMP:
                    emit_py(c + 2)
                z = zp.tile([COMP, S], BF16, tag="z", name="z")
                nc.vector.tensor_tensor(z[:], pys.pop(c)[:], d2[:], OP.mult)
                # pr in two 1-bank chunks (own bufs) so the sq reader never
                # serializes the next pr; squares scaled into fp8 pair tiles
                if ACC_DR:
                    if c % 2 == 0:
                        sq2 = sqp.tile([COMP, 2, S], FP8, tag="sq",
                                       name="sq2")
                        half = 0
                    else:
                        sq2 = pend[-1]
                        half = 1
                else:
                    sq2 = sqp.tile([COMP, S], BF16, tag="sq", name="sq2")
                for i0 in range(0, S, 512):
                    sl = slice(i0, i0 + 512)
                    pr = psPr.tile([COMP, 512], F32, tag="pr", name="pr")
                    nc.tensor.matmul(pr[0:COMP, :], w["mw3Tb"], z[:, sl],
                                     start=True, stop=True)
                    dst = sq2[:, half, sl] if ACC_DR else sq2[:, sl]
                    nc.scalar.activation(dst, pr[0:COMP, :], AF.Square,
                                         scale=SQS if ACC_DR else 1.0)
                if ACC_DR:
                    if c % 2 == 0:
                        pend.append(sq2)
                    # defer pair-acc one pair so PE never heads-of-line
                    # blocks the mask chain
                    if c % 2 == 1 and len(pend) > 1:
                        emit_acc_pair(pend.pop(0)[:], last=False)
                else:
                    pend.append(sq2)
                    while len(pend) > 2 or (c == COMP - 1 and pend):
                        sqd = pend.pop(0)
                        for sl in jchunks():
                            nc.tensor.matmul(
                                accp[0:COMP, sl], w["idenb"], sqd[:, sl],
                                start=(acc_n[0] == 0),
                                stop=(acc_n[0] == COMP - 1),
                                skip_group_check=True)
                        acc_n[0] += 1
            if ACC_DR:
                # flush: one full pair + the final odd c (half 1 zeroed)
                last_sq2 = pend.pop()          # c=84, half 0 written
                nc.gpsimd.memset(last_sq2[0:COMP, 1, :], 0.0)
                while pend:
                    emit_acc_pair(pend.pop(0)[:], last=False)
                emit_acc_pair(last_sq2[:], last=True)

            # ---- finale ----
            # jn2' = SQS^2 * jn2 lands in jn2k row 0 next to kout (row 1)
            am = zp.tile([COMP, S], BF16, tag="z", name="am")
            nc.vector.tensor_tensor(am[:], accp[0:COMP, :], d3[:], OP.mult)
            pj = psBig.tile([1, S], F32, tag="big", name="pj")
            for sl in jchunks():
                nc.tensor.matmul(pj[0:1, sl], w["onesb"], am[:, sl],
                                 start=True, stop=True)
            jn2 = ap_.tile([1, S], F32, tag="jn2", name="jn2")
            nc.scalar.copy(jn2[:], pj[0:1, :])

            # transpose jn2' / kout into per-sample layout [128, 2*NB]
            pjk = psPr.tile([128, 2 * NB], F32, tag="pr", name="pjk")
            for nb in range(NB):
                sl = slice(nb * 128, (nb + 1) * 128)
                nc.tensor.transpose(pjk[:, nb:nb + 1], jn2[:, sl],
                                    w["iden"][0:1, 0:1])
                nc.tensor.transpose(pjk[:, NB + nb:NB + nb + 1], kout[:, sl],
                                    w["iden"][0:1, 0:1])
            pjs = ap_.tile([128, 2 * NB], F32, tag="pjs", name="pjs")
            nc.scalar.copy(pjs[:], pjk[:])
            x = pjs[:, 0:NB]
            koutT = pjs[:, NB:2 * NB]
            sqs = SQS if ACC_DR else 1.0

            scl = smp.tile([128, NB], F32, tag="scl", name="scl")
            if USE_QUAKE:
                # rsqrt(x) via Quake init + Newton steps, all on DVE
                xi = x.bitcast(I32)
                t1 = smp.tile([128, NB], I32, tag="t1", name="t1")
                nc.vector.tensor_scalar(out=t1[:], in0=xi, scalar1=1,
                                        scalar2=None,
                                        op0=OP.arith_shift_right)
                t2 = smp.tile([128, NB], I32, tag="t2", name="t2")
                nc.vector.tensor_scalar(out=t2[:], in0=t1[:],
                                        scalar1=float(MAGIC), scalar2=-1.0,
                                        op0=OP.subtract, op1=OP.mult)
                y = t2[:].bitcast(F32)
                tmp = smp.tile([128, NB], F32, tag="tmp", name="tmp")
                for _ in range(NEWTON):
                    nc.vector.tensor_tensor(tmp[:], y, y, OP.mult)
                    nc.vector.tensor_tensor(tmp[:], tmp[:], x, OP.mult)
                    nc.vector.tensor_scalar(out=tmp[:], in0=tmp[:],
                                            scalar1=-0.5, scalar2=1.5,
                                            op0=OP.mult, op1=OP.add)
                    nc.vector.tensor_tensor(t2[:].bitcast(F32), y, tmp[:],
                                            OP.mult)
                den = smp.tile([128, NB], F32, tag="den", name="den")
                nc.vector.tensor_tensor(den[:], x, y, OP.mult)  # = sqrt(x)
            else:
                den = smp.tile([128, NB], F32, tag="den", name="den")
                nc.scalar.activation(den[:], x, AF.Sqrt)
            # scl = sqs*kout / (den + sqs*EPS)   (den = sqs*jn)
            rec = smp.tile([128, NB], F32, tag="rec", name="rec")
            nc.vector.tensor_scalar_add(rec[:], den[:], sqs * EPS)
            nc.vector.reciprocal(rec[:], rec[:])
            nc.vector.tensor_tensor(scl[:], rec[:], koutT, OP.mult)
            if sqs != 1.0:
                nc.vector.tensor_scalar_mul(scl[:], scl[:], sqs)

            ot = outp.tile([128, NB, COMP], F32, tag="ot", name="ot")
            for nb in range(NB):
                nc.scalar.activation(ot[:, nb, :], foutT[:, nb, :], AF.Tanh,
                                     scale=scl[:, nb:nb + 1])
            nc.sync.dma_start(out_d[:].rearrange("(nb p) f -> p nb f", p=128),
                              ot[:])

    return nc


_NC = None


def _get_nc():
    global _NC
    if _NC is None:
        _NC = build_nc()
        _NC.finalize()
    return _NC


def make_in_maps(inputs):
    w = host_prep(inputs)
    obs = np.ascontiguousarray(np.asarray(inputs["obs"], np.float32))
    act = np.ascontiguousarray(np.asarray(inputs["action"], np.float32))
    in_maps = []
    for i in range(NCORES):
        m = dict(w)
        m["obs"] = np.ascontiguousarray(obs[i * S:(i + 1) * S])
        m["action"] = np.ascontiguousarray(act[i * S:(i + 1) * S])
        in_maps.append(m)
    return in_maps


def kernel(**inputs):
    from concourse.bass_utils import run_bass_kernel_spmd

    nc = _get_nc()
    in_maps = make_in_maps(inputs)
    res = run_bass_kernel_spmd(nc, in_maps, core_ids=list(range(NCORES)))
    return np.concatenate([r["out"] for r in res.results], axis=0)
